# revision 20
# baseline (speedup 1.0000x reference)
"""Trainium2 Bass kernel for nn_Attention2 (dense transformer block with
softmax over the heads axis).

Computation per (n, t) batch b (B = n*t = 4096 total, X_b = x[n,:,t,:].T is
[vv=25, c=512]):
    qkv = X_b @ w_qkv.T, split into q,k,v heads [h=8, 25, hd=64]
    s[h,i,j] = (q[h,i,:] . k[h,j,:]) / 8      (scale folded into w_q on host)
    p = softmax over h (axis 0)
    o[h,i,:] = sum_j p[h,i,j] v[h,j,:]  -> [25, 512] -> @ w_proj.T
    out[n,:,t,:] = result.T

Sharding: data-parallel over n, 2 n-values (512 batches) per core, 8 cores.

v2 design (~592us vs 902us baseline; PE-matmul-bound, ~94% PE occupancy):
 - 3-stage software pipeline per emission iteration: qk+v GEMMs of group g,
   attention (s-matmuls, softmax, o-matmuls) of group g-1, proj of group
   g-2.  The PE queue never stalls on the softmax chain, so the HAM clock
   gate stays warm (the old kernel oscillated at half clock all run).
 - dense contiguous moving operands for qk/proj (separate dense x copy),
   contiguous PSUM evacuations, evac work split across ACT and DVE, softmax
   p-multiply on GpSimd; softmax head-sum as a single strided-view reduce.
 - o-matmul PSUM: one bank per batch-row-group.  Matmuls whose
   tile_position rows differ run CONCURRENTLY on the PE sub-arrays, and
   concurrent drains into one PSUM bank are a fatal collision - every bank
   may only be written from a single tile_position row value.
 - everything f16 (fp8 DoubleRow fails the 2e-2 gate: ~4-7% rel err from
   e4m3 quantization; measured on-host).  f16 keeps rel err at 6.9e-4.
"""
import os
import numpy as np
import concourse.bass as bass
import concourse.mybir as mybir
import concourse.tile as tile
from concourse.bass_utils import run_bass_kernel_spmd
from concourse.vector_clock import ScopedClock, VectorClock

F32 = mybir.dt.float32
F16 = mybir.dt.float16

N_CORES = 8
NN_PER_CORE = 2        # n values per core
T = 256
VV = 25
C = 512
H = 8
HD = 64
TG = 16                # t values (batches) per group
NG = NN_PER_CORE * (T // TG)   # 32 groups per core
NGRUN = NG
NB = TG * VV           # 400 moving columns per group


def _split_drain_and_barrier(self, tick_clock, wait_clock):
    # walrus caps sync-wait commands at 1 for CTRL_NO; split the kernel-tail
    # drain into one drain per pending proc.
    vc = tick_clock.global_clock
    n = len(vc)
    for i in range(n):
        if vc[i] == 0:
            continue
        sub = VectorClock([vc[j] if j == i else 0 for j in range(n)])
        d = self.nc.sync.drain()
        wait_clock.add_sem_waits(d.ins, ScopedClock({None: sub}))
    self.nc.all_engine_barrier()
    assert self.sems is not None
    popped = self.nc._tile_sem_poison_stack.pop()
    assert popped is self._sem_poison
    self.nc.clear_and_free_semaphores(list(self.sems.allocated().values()))
    self.nc.all_engine_barrier()


tile.TileContext._drain_and_barrier = _split_drain_and_barrier


def split_excess_waits(nc, limit=1):
    """walrus codegen allows very few sync-wait commands per instruction
    (1 for matmul/drain/DMA structs).  Move excess waits onto same-engine
    NoOp carriers inserted just before the instruction — same semantics,
    since each engine executes its queue in order."""
    k = 0
    for fn in nc.m.functions:
        for bb in fn.blocks:
            out = []
            for ins in bb.instructions:
                si = ins.sync_info
                waits = list(si.on_wait) if si is not None and si.on_wait else []
                if len(waits) > limit:
                    keep = waits[-limit:]
                    for w in waits[:-limit]:
                        nop = mybir.InstNoOp(
                            name=f"WC-{k}", ins=[], outs=[], engine=ins.engine
                        )
                        k += 1
                        nop.sync_info = mybir.SyncInfo(on_wait=[w], on_update=[])
                        out.append(nop)
                    si.on_wait = keep
                out.append(ins)
            bb.instructions[:] = out
    return k


def build_nc():

    nc = bass.Bass()
    X = nc.declare_dram_parameter("x", [NN_PER_CORE, C, T, VV], F16, isOutput=False)
    WQK = nc.declare_dram_parameter("wqkT", [C, 2 * C], F16, isOutput=False)
    WV = nc.declare_dram_parameter("wvT", [C, C], F16, isOutput=False)
    WP = nc.declare_dram_parameter("wprojT", [C, C], F16, isOutput=False)
    Y = nc.declare_dram_parameter("y", [NN_PER_CORE, C, T, VV], F16, isOutput=True)

    with tile.TileContext(nc) as tc:
        with (
            tc.tile_pool(name="consts", bufs=1) as consts,
            tc.tile_pool(name="xpool", bufs=2) as xpool,
            tc.tile_pool(name="qkpool", bufs=2) as qkpool,
            tc.tile_pool(name="vpool", bufs=2) as vpool,
            tc.tile_pool(name="softpool", bufs=2) as softpool,
            tc.tile_pool(name="otpool", bufs=3) as otpool,
            tc.tile_pool(name="finpool", bufs=4) as finpool,
            tc.tile_pool(name="pbig", bufs=2, space="PSUM") as pbig,
            tc.tile_pool(name="ppsm", bufs=1, space="PSUM") as ppsm,
            tc.tile_pool(name="ppo", bufs=1, space="PSUM") as ppo,
        ):
            # ---- weights ----
            wqk_r, wv_r, wp_r = [], [], []
            for kc in range(4):
                r0 = consts.tile([128, 2 * C], F16, tag=f"wqkr{kc}", name=f"wqkr{kc}")
                nc.sync.dma_start(out=r0, in_=WQK[kc * 128:(kc + 1) * 128, :])
                wqk_r.append(r0)
            for kc in range(4):
                r1 = consts.tile([128, C], F16, tag=f"wvr{kc}", name=f"wvr{kc}")
                nc.sync.dma_start(out=r1, in_=WV[kc * 128:(kc + 1) * 128, :])
                wv_r.append(r1)
            for kc in range(4):
                r2 = consts.tile([128, C], F16, tag=f"wpr{kc}", name=f"wpr{kc}")
                nc.sync.dma_start(out=r2, in_=WP[kc * 128:(kc + 1) * 128, :])
                wp_r.append(r2)

            # per-group tile getters (tag-rotated by the pools)
            def load_xp(g):
                nn = g // (T // TG)
                t0 = (g % (T // TG)) * TG
                xp, xd = [], []
                for kc in range(4):
                    xq = xpool.tile([128, TG, 32], F16, tag=f"xp{kc}", name=f"xp{kc}")
                    nc.sync.dma_start(
                        out=xq[:, :, 0:VV],
                        in_=X[nn, kc * 128:(kc + 1) * 128, t0:t0 + TG, :],
                    )
                    xp.append(xq)
                    xq2 = xpool.tile([128, NB], F16, tag=f"xd{kc}", name=f"xd{kc}")
                    nc.sync.dma_start(
                        out=xq2[:].rearrange("p (t v) -> p t v", t=TG),
                        in_=X[nn, kc * 128:(kc + 1) * 128, t0:t0 + TG, :],
                    )
                    xd.append(xq2)
                return xp, xd

            xp_of = {}       # g -> xp tiles
            qkT_of = {}      # g -> 8 qkT tiles ([128, TG, 32] f16)
            vsb_of = {}      # g -> 4 v_sb tiles ([128, C] f16)
            p2_of = {}       # g -> 4 p2 tiles ([128, 2, 4, VV] f16)
            oT_of = {}       # g -> oT tile ([128, 4, TG, 32] f16)

            xp_of[0] = load_xp(0)  # (xp, xd)

            for it in range(NGRUN + 2):
                g1 = it          # GEMM stage
                g0 = it - 1      # attention stage
                gp = it - 2      # proj stage

                if g1 + 1 < NGRUN:
                    xp_of[g1 + 1] = load_xp(g1 + 1)

                # ---------- attention psm (g0) interleaved with qk GEMM (g1) ----
                att = 0 <= g0 < NGRUN
                if att:
                    qkT = qkT_of[g0]
                    psm = [
                        ppsm.tile([128, 4, VV], F32, tag=f"psm{par}", name=f"psm{par}")
                        for par in range(2)
                    ]
                    p2_l = []

                if g1 < NGRUN:
                    xp, xd = xp_of[g1]
                    qkT_new = [
                        qkpool.tile([128, NB], F16, tag=f"qkT{m}", name=f"qkT{m}")
                        for m in range(8)
                    ]
                    qkT_of[g1] = qkT_new

                # interleave: psm bundles for sub s, then 2 qk m-chunks
                for step in range(4):
                    if att:
                        sub = step
                        # s-matmul bundles: per head-pair m one LDW covering
                        # 4 batches x 2 heads
                        for m in range(4):
                            for b4 in range(4):
                                for par in range(2):
                                    slot = 4 * sub + b4
                                    mm = nc.tensor.matmul(
                                        psm[par][32 * b4:32 * b4 + VV, m, :],
                                        qkT[4 + m][64 * par:64 * par + 64,
                                                   slot * VV:(slot + 1) * VV],
                                        qkT[m][64 * par:64 * par + 64,
                                               slot * VV:(slot + 1) * VV],
                                        start=True, stop=True,
                                        tile_position=(64 * par, 32 * b4),
                                    )
                        # softmax for this sub (ACT exp, DVE reduce/recip/mul)
                        e_t = softpool.tile([128, 2, 4, VV], F32, tag="e_t", name="e_t", bufs=4)
                        for par in range(2):
                            nc.scalar.activation(
                                e_t[:, par, :, :], psm[par][:],
                                mybir.ActivationFunctionType.Exp,
                            )
                        D = softpool.tile([128, VV], F32, tag="D", name="D")
                        nc.vector.reduce_sum(
                            out=D[:],
                            in_=e_t[:].rearrange("p a m i -> p i (a m)"),
                            axis=mybir.AxisListType.X,
                        )
                        rD = softpool.tile([128, VV], F32, tag="rD", name="rD")
                        nc.vector.reciprocal(rD[:], D[:])
                        p2 = softpool.tile([128, 2, 4, VV], F16, tag="p2", name="p2", bufs=4)
                        nc.gpsimd.tensor_mul(
                            p2[:],
                            e_t[:],
                            rD[:].unsqueeze(1).unsqueeze(1)
                                .broadcast_to([128, 2, 4, VV]),
                        )
                        p2_l.append(p2)

                    if g1 < NGRUN:
                        for m in (2 * step, 2 * step + 1):
                            pq = pbig.tile([128, NB], F32, tag="big", name="pq")
                            for kc in range(4):
                                nc.tensor.matmul(
                                    pq[:],
                                    wqk_r[kc][:, m * 128:(m + 1) * 128],
                                    xd[kc][:],
                                    start=(kc == 0), stop=(kc == 3),
                                )
                            # contiguous evacuation; alternate engines
                            dst = qkT_new[m][:]
                            src = pq[:]
                            if m % 2 == 0:
                                nc.vector.tensor_copy(dst, src)
                            else:
                                nc.scalar.activation(
                                    dst, src, mybir.ActivationFunctionType.Copy,
                                )

                if att:
                    p2_of[g0] = p2_l

                # ---------- v GEMM (g1) interleaved with po bundles (g0) ------
                if att:
                    oT = otpool.tile([128, 4, NB], F16, tag="oT", name="oT")
                    oT_of[g0] = oT
                for sub in range(4):
                    if g1 < NGRUN:
                        xp, xd = xp_of[g1]
                        pv = pbig.tile([128, C], F32, tag="big", name="pv")
                        for kc in range(4):
                            nc.tensor.matmul(
                                pv[:],
                                xp[kc][:, 4 * sub:4 * sub + 4, :],
                                wv_r[kc][:],
                                start=(kc == 0), stop=(kc == 3),
                            )
                        v_sb = vpool.tile([128, C], F16, tag=f"v{sub}", name=f"v{sub}")
                        nc.vector.tensor_copy(v_sb[:, 0:256], pv[:, 0:256])
                        nc.scalar.activation(
                            v_sb[:, 256:512], pv[:, 256:512],
                            mybir.ActivationFunctionType.Copy,
                        )
                        vsb_of.setdefault(g1, []).append(v_sb)

                    if att:
                        v_sb0 = vsb_of[g0][sub]
                        p2 = p2_of[g0][sub]
                        po = [
                            ppo.tile([128, 4, VV], F32, tag=f"po{b4}", name=f"po{b4}")
                            for b4 in range(4)
                        ]
                        for mp in range(4):
                            for e in range(2):
                                for b4 in range(4):
                                    mm = nc.tensor.matmul(
                                        po[b4][64 * e:64 * e + 64, mp, :],
                                        v_sb0[32 * b4:32 * b4 + VV,
                                              128 * mp + 64 * e:128 * mp + 64 * e + 64],
                                        p2[32 * b4:32 * b4 + VV, e, mp, :],
                                        start=True, stop=True,
                                        tile_position=(32 * b4, 64 * e),
                                    )
                        # evacuate po -> oT slots (DVE)
                        oT0 = oT_of[g0]
                        oT0v = oT0[:].rearrange("p m (s v) -> p m s v", v=VV)
                        for b4 in range(4):
                            if b4 % 2 == 0:
                                nc.vector.tensor_copy(
                                    oT0v[:, :, 4 * sub + b4, :], po[b4][:],
                                )
                            else:
                                nc.scalar.activation(
                                    oT0v[:, :, 4 * sub + b4, :], po[b4][:],
                                    mybir.ActivationFunctionType.Copy,
                                )

                # ---------- proj (gp) ----------------------------------------
                if 0 <= gp < NGRUN:
                    nn = gp // (T // TG)
                    t0 = (gp % (T // TG)) * TG
                    oTp = oT_of.pop(gp)
                    for co in range(4):
                        pf = pbig.tile([128, NB], F32, tag="big", name="pf")
                        for kc in range(4):
                            nc.tensor.matmul(
                                pf[:],
                                wp_r[kc][:, co * 128:(co + 1) * 128],
                                oTp[:, kc, :],
                                start=(kc == 0), stop=(kc == 3),
                            )
                        fin = finpool.tile([128, NB], F16, tag="fin", name="fin")
                        nc.scalar.activation(
                            fin[:], pf[:], mybir.ActivationFunctionType.Copy,
                        )
                        nc.sync.dma_start(
                            out=Y[nn, co * 128:(co + 1) * 128, t0:t0 + TG, :],
                            in_=fin[:].rearrange("p (t v) -> p t v", t=TG),
                        )
                    # drop references for dead groups
                    if gp in xp_of:
                        del xp_of[gp]
                    qkT_of.pop(gp, None)
                    vsb_of.pop(gp, None)
                    p2_of.pop(gp, None)

    return nc


LAST_RESULT = {}


def kernel(x: np.ndarray, w_qkv: np.ndarray, w_proj: np.ndarray,
           _trace: bool = False) -> np.ndarray:
    n, c, t, vv = x.shape
    assert (n, c, t, vv) == (16, 512, 256, 25)
    scale = np.float32((c // H) ** -0.5)

    wq = w_qkv[:c] * scale
    wk = w_qkv[c:2 * c]
    wv = w_qkv[2 * c:]
    wqkT = np.ascontiguousarray(np.concatenate([wq, wk], axis=0).T.astype(np.float16))
    wvT = np.ascontiguousarray(wv.T.astype(np.float16))
    wprojT = np.ascontiguousarray(w_proj.T.astype(np.float16))

    nc = build_nc()
    split_excess_waits(nc)
    in_maps = []
    for core in range(N_CORES):
        shard = np.ascontiguousarray(
            x[core * NN_PER_CORE:(core + 1) * NN_PER_CORE].astype(np.float16)
        )
        in_maps.append({"x": shard, "wqkT": wqkT, "wvT": wvT, "wprojT": wprojT})

    kw = {}
    if _trace:
        import tempfile
        kw = dict(trace=True, tmpdir=tempfile.mkdtemp(prefix="attn2_trace_"))
    res = run_bass_kernel_spmd(nc, in_maps, list(range(N_CORES)), **kw)
    LAST_RESULT["res"] = res
    LAST_RESULT["tmpdir"] = kw.get("tmpdir")
    out = np.empty((n, c, t, vv), dtype=np.float32)
    for core in range(N_CORES):
        out[core * NN_PER_CORE:(core + 1) * NN_PER_CORE] = \
            res.results[core]["y"].astype(np.float32)
    return out


# revision 21
# speedup vs baseline: 1.0122x; 1.0122x over previous
"""Trainium2 Bass kernel for nn_Attention2 (dense transformer block with
softmax over the heads axis).

Computation per (n, t) batch b (B = n*t = 4096 total, X_b = x[n,:,t,:].T is
[vv=25, c=512]):
    qkv = X_b @ w_qkv.T, split into q,k,v heads [h=8, 25, hd=64]
    s[h,i,j] = (q[h,i,:] . k[h,j,:]) / 8      (scale folded into w_q on host)
    p = softmax over h (axis 0)
    o[h,i,:] = sum_j p[h,i,j] v[h,j,:]  -> [25, 512] -> @ w_proj.T
    out[n,:,t,:] = result.T

Sharding: data-parallel over n, 2 n-values (512 batches) per core, 8 cores.

v2 design (~592us vs 902us baseline; PE-matmul-bound, ~94% PE occupancy):
 - 3-stage software pipeline per emission iteration: qk+v GEMMs of group g,
   attention (s-matmuls, softmax, o-matmuls) of group g-1, proj of group
   g-2.  The PE queue never stalls on the softmax chain, so the HAM clock
   gate stays warm (the old kernel oscillated at half clock all run).
 - dense contiguous moving operands for qk/proj (separate dense x copy),
   contiguous PSUM evacuations, evac work split across ACT and DVE, softmax
   p-multiply on GpSimd; softmax head-sum as a single strided-view reduce.
 - o-matmul PSUM: one bank per batch-row-group.  Matmuls whose
   tile_position rows differ run CONCURRENTLY on the PE sub-arrays, and
   concurrent drains into one PSUM bank are a fatal collision - every bank
   may only be written from a single tile_position row value.
 - everything f16 (fp8 DoubleRow fails the 2e-2 gate: ~4-7% rel err from
   e4m3 quantization; measured on-host).  f16 keeps rel err at 6.9e-4.
"""
import os
import numpy as np
import concourse.bass as bass
import concourse.mybir as mybir
import concourse.tile as tile
from concourse.bass_utils import run_bass_kernel_spmd
from concourse.vector_clock import ScopedClock, VectorClock

F32 = mybir.dt.float32
F16 = mybir.dt.float16

N_CORES = 8
NN_PER_CORE = 2        # n values per core
T = 256
VV = 25
C = 512
H = 8
HD = 64
TG = 16                # t values (batches) per group
NG = NN_PER_CORE * (T // TG)   # 32 groups per core
NGRUN = NG
NB = TG * VV           # 400 moving columns per group


def _split_drain_and_barrier(self, tick_clock, wait_clock):
    # walrus caps sync-wait commands at 1 for CTRL_NO; split the kernel-tail
    # drain into one drain per pending proc.
    vc = tick_clock.global_clock
    n = len(vc)
    for i in range(n):
        if vc[i] == 0:
            continue
        sub = VectorClock([vc[j] if j == i else 0 for j in range(n)])
        d = self.nc.sync.drain()
        wait_clock.add_sem_waits(d.ins, ScopedClock({None: sub}))
    self.nc.all_engine_barrier()
    assert self.sems is not None
    popped = self.nc._tile_sem_poison_stack.pop()
    assert popped is self._sem_poison
    self.nc.clear_and_free_semaphores(list(self.sems.allocated().values()))
    self.nc.all_engine_barrier()


tile.TileContext._drain_and_barrier = _split_drain_and_barrier


def split_excess_waits(nc, limit=1):
    """walrus codegen allows very few sync-wait commands per instruction
    (1 for matmul/drain/DMA structs).  Move excess waits onto same-engine
    NoOp carriers inserted just before the instruction — same semantics,
    since each engine executes its queue in order."""
    k = 0
    for fn in nc.m.functions:
        for bb in fn.blocks:
            out = []
            for ins in bb.instructions:
                si = ins.sync_info
                waits = list(si.on_wait) if si is not None and si.on_wait else []
                if len(waits) > limit:
                    keep = waits[-limit:]
                    for w in waits[:-limit]:
                        nop = mybir.InstNoOp(
                            name=f"WC-{k}", ins=[], outs=[], engine=ins.engine
                        )
                        k += 1
                        nop.sync_info = mybir.SyncInfo(on_wait=[w], on_update=[])
                        out.append(nop)
                    si.on_wait = keep
                out.append(ins)
            bb.instructions[:] = out
    return k


def build_nc():

    nc = bass.Bass()
    X = nc.declare_dram_parameter("x", [NN_PER_CORE, C, T, VV], F16, isOutput=False)
    WQK = nc.declare_dram_parameter("wqkT", [C, 2 * C], F16, isOutput=False)
    WV = nc.declare_dram_parameter("wvT", [C, C], F16, isOutput=False)
    WP = nc.declare_dram_parameter("wprojT", [C, C], F16, isOutput=False)
    Y = nc.declare_dram_parameter("y", [NN_PER_CORE, C, T, VV], F16, isOutput=True)

    with tile.TileContext(nc) as tc:
        with (
            tc.tile_pool(name="consts", bufs=1) as consts,
            tc.tile_pool(name="xpool", bufs=2) as xpool,
            tc.tile_pool(name="qkpool", bufs=2) as qkpool,
            tc.tile_pool(name="vpool", bufs=2) as vpool,
            tc.tile_pool(name="softpool", bufs=2) as softpool,
            tc.tile_pool(name="otpool", bufs=3) as otpool,
            tc.tile_pool(name="finpool", bufs=4) as finpool,
            tc.tile_pool(name="pbig", bufs=2, space="PSUM") as pbig,
            tc.tile_pool(name="ppsm", bufs=1, space="PSUM") as ppsm,
            tc.tile_pool(name="ppo", bufs=1, space="PSUM") as ppo,
        ):
            # ---- weights ----
            wqk_r, wv_r, wp_r = [], [], []
            for kc in range(4):
                r0 = consts.tile([128, 2 * C], F16, tag=f"wqkr{kc}", name=f"wqkr{kc}")
                nc.sync.dma_start(out=r0, in_=WQK[kc * 128:(kc + 1) * 128, :])
                wqk_r.append(r0)
            for kc in range(4):
                r1 = consts.tile([128, C], F16, tag=f"wvr{kc}", name=f"wvr{kc}")
                nc.sync.dma_start(out=r1, in_=WV[kc * 128:(kc + 1) * 128, :])
                wv_r.append(r1)
            for kc in range(4):
                r2 = consts.tile([128, C], F16, tag=f"wpr{kc}", name=f"wpr{kc}")
                nc.sync.dma_start(out=r2, in_=WP[kc * 128:(kc + 1) * 128, :])
                wp_r.append(r2)

            # per-group tile getters (tag-rotated by the pools)
            def load_xp(g):
                nn = g // (T // TG)
                t0 = (g % (T // TG)) * TG
                xp, xd = [], []
                for kc in range(4):
                    xq = xpool.tile([128, TG, 32], F16, tag=f"xp{kc}", name=f"xp{kc}")
                    nc.sync.dma_start(
                        out=xq[:, :, 0:VV],
                        in_=X[nn, kc * 128:(kc + 1) * 128, t0:t0 + TG, :],
                    )
                    xp.append(xq)
                    xq2 = xpool.tile([128, NB], F16, tag=f"xd{kc}", name=f"xd{kc}")
                    nc.sync.dma_start(
                        out=xq2[:].rearrange("p (t v) -> p t v", t=TG),
                        in_=X[nn, kc * 128:(kc + 1) * 128, t0:t0 + TG, :],
                    )
                    xd.append(xq2)
                return xp, xd

            xp_of = {}       # g -> xp tiles
            qkT_of = {}      # g -> 8 qkT tiles ([128, TG, 32] f16)
            vsb_of = {}      # g -> 4 v_sb tiles ([128, C] f16)
            p2_of = {}       # g -> 4 p2 tiles ([128, 2, 4, VV] f16)
            oT_of = {}       # g -> oT tile ([128, 4, TG, 32] f16)

            xp_of[0] = load_xp(0)  # (xp, xd)

            for it in range(NGRUN + 2):
                g1 = it          # GEMM stage
                g0 = it - 1      # attention stage
                gp = it - 2      # proj stage

                if g1 + 1 < NGRUN:
                    xp_of[g1 + 1] = load_xp(g1 + 1)

                # ---------- attention psm (g0) interleaved with qk GEMM (g1) ----
                att = 0 <= g0 < NGRUN
                if att:
                    qkT = qkT_of[g0]
                    psm = [
                        ppsm.tile([128, 4, VV], F32, tag=f"psm{par}", name=f"psm{par}")
                        for par in range(2)
                    ]
                    p2_l = []

                if g1 < NGRUN:
                    xp, xd = xp_of[g1]
                    qkT_new = [
                        qkpool.tile([128, NB], F16, tag=f"qkT{m}", name=f"qkT{m}")
                        for m in range(8)
                    ]
                    qkT_of[g1] = qkT_new

                # interleave: psm bundles for sub s, then 2 qk m-chunks
                for step in range(4):
                    if att:
                        sub = step
                        # s-matmul bundles: per head-pair m one LDW covering
                        # 4 batches x 2 heads
                        for m in range(4):
                            for par in range(2):
                                for b4 in range(4):
                                    slot = 4 * sub + b4
                                    mm = nc.tensor.matmul(
                                        psm[par][32 * b4:32 * b4 + VV, m, :],
                                        qkT[4 + m][64 * par:64 * par + 64,
                                                   slot * VV:(slot + 1) * VV],
                                        qkT[m][64 * par:64 * par + 64,
                                               slot * VV:(slot + 1) * VV],
                                        start=True, stop=True,
                                        tile_position=(64 * par, 32 * b4),
                                    )
                        # softmax for this sub (ACT exp, DVE reduce/recip/mul)
                        e_t = softpool.tile([128, 2, 4, VV], F32, tag="e_t", name="e_t", bufs=4)
                        for par in range(2):
                            nc.scalar.activation(
                                e_t[:, par, :, :], psm[par][:],
                                mybir.ActivationFunctionType.Exp,
                            )
                        D = softpool.tile([128, VV], F32, tag="D", name="D")
                        nc.vector.reduce_sum(
                            out=D[:],
                            in_=e_t[:].rearrange("p a m i -> p i (a m)"),
                            axis=mybir.AxisListType.X,
                        )
                        rD = softpool.tile([128, VV], F32, tag="rD", name="rD")
                        nc.vector.reciprocal(rD[:], D[:])
                        p2 = softpool.tile([128, 2, 4, VV], F16, tag="p2", name="p2", bufs=4)
                        nc.gpsimd.tensor_mul(
                            p2[:],
                            e_t[:],
                            rD[:].unsqueeze(1).unsqueeze(1)
                                .broadcast_to([128, 2, 4, VV]),
                        )
                        p2_l.append(p2)

                    if g1 < NGRUN:
                        for m in (2 * step, 2 * step + 1):
                            pq = pbig.tile([128, NB], F32, tag="big", name="pq")
                            for kc in range(4):
                                nc.tensor.matmul(
                                    pq[:],
                                    wqk_r[kc][:, m * 128:(m + 1) * 128],
                                    xd[kc][:],
                                    start=(kc == 0), stop=(kc == 3),
                                )
                            # contiguous evacuation; alternate engines
                            dst = qkT_new[m][:]
                            src = pq[:]
                            if m % 2 == 0:
                                nc.vector.tensor_copy(dst, src)
                            else:
                                nc.scalar.activation(
                                    dst, src, mybir.ActivationFunctionType.Copy,
                                )

                if att:
                    p2_of[g0] = p2_l

                # ---------- v GEMM (g1) interleaved with po bundles (g0) ------
                if att:
                    oT = otpool.tile([128, 4, NB], F16, tag="oT", name="oT")
                    oT_of[g0] = oT
                for sub in range(4):
                    if g1 < NGRUN:
                        xp, xd = xp_of[g1]
                        pv = pbig.tile([128, C], F32, tag="big", name="pv")
                        for kc in range(4):
                            nc.tensor.matmul(
                                pv[:],
                                xp[kc][:, 4 * sub:4 * sub + 4, :],
                                wv_r[kc][:],
                                start=(kc == 0), stop=(kc == 3),
                            )
                        v_sb = vpool.tile([128, C], F16, tag=f"v{sub}", name=f"v{sub}")
                        nc.vector.tensor_copy(v_sb[:, 0:256], pv[:, 0:256])
                        nc.scalar.activation(
                            v_sb[:, 256:512], pv[:, 256:512],
                            mybir.ActivationFunctionType.Copy,
                        )
                        vsb_of.setdefault(g1, []).append(v_sb)

                    if att:
                        v_sb0 = vsb_of[g0][sub]
                        p2 = p2_of[g0][sub]
                        po = [
                            ppo.tile([128, 4, VV], F32, tag=f"po{b4}", name=f"po{b4}")
                            for b4 in range(4)
                        ]
                        for mp in range(4):
                            for e in range(2):
                                for b4 in range(4):
                                    mm = nc.tensor.matmul(
                                        po[b4][64 * e:64 * e + 64, mp, :],
                                        v_sb0[32 * b4:32 * b4 + VV,
                                              128 * mp + 64 * e:128 * mp + 64 * e + 64],
                                        p2[32 * b4:32 * b4 + VV, e, mp, :],
                                        start=True, stop=True,
                                        tile_position=(32 * b4, 64 * e),
                                    )
                        # evacuate po -> oT slots (DVE)
                        oT0 = oT_of[g0]
                        oT0v = oT0[:].rearrange("p m (s v) -> p m s v", v=VV)
                        for b4 in range(4):
                            if b4 % 2 == 0:
                                nc.vector.tensor_copy(
                                    oT0v[:, :, 4 * sub + b4, :], po[b4][:],
                                )
                            else:
                                nc.scalar.activation(
                                    oT0v[:, :, 4 * sub + b4, :], po[b4][:],
                                    mybir.ActivationFunctionType.Copy,
                                )

                # ---------- proj (gp) ----------------------------------------
                if 0 <= gp < NGRUN:
                    nn = gp // (T // TG)
                    t0 = (gp % (T // TG)) * TG
                    oTp = oT_of.pop(gp)
                    for co in range(4):
                        pf = pbig.tile([128, NB], F32, tag="big", name="pf")
                        for kc in range(4):
                            nc.tensor.matmul(
                                pf[:],
                                wp_r[kc][:, co * 128:(co + 1) * 128],
                                oTp[:, kc, :],
                                start=(kc == 0), stop=(kc == 3),
                            )
                        fin = finpool.tile([128, NB], F16, tag="fin", name="fin")
                        nc.scalar.activation(
                            fin[:], pf[:], mybir.ActivationFunctionType.Copy,
                        )
                        nc.sync.dma_start(
                            out=Y[nn, co * 128:(co + 1) * 128, t0:t0 + TG, :],
                            in_=fin[:].rearrange("p (t v) -> p t v", t=TG),
                        )
                    # drop references for dead groups
                    if gp in xp_of:
                        del xp_of[gp]
                    qkT_of.pop(gp, None)
                    vsb_of.pop(gp, None)
                    p2_of.pop(gp, None)

    return nc


LAST_RESULT = {}


def kernel(x: np.ndarray, w_qkv: np.ndarray, w_proj: np.ndarray,
           _trace: bool = False) -> np.ndarray:
    n, c, t, vv = x.shape
    assert (n, c, t, vv) == (16, 512, 256, 25)
    scale = np.float32((c // H) ** -0.5)

    wq = w_qkv[:c] * scale
    wk = w_qkv[c:2 * c]
    wv = w_qkv[2 * c:]
    wqkT = np.ascontiguousarray(np.concatenate([wq, wk], axis=0).T.astype(np.float16))
    wvT = np.ascontiguousarray(wv.T.astype(np.float16))
    wprojT = np.ascontiguousarray(w_proj.T.astype(np.float16))

    nc = build_nc()
    split_excess_waits(nc)
    in_maps = []
    for core in range(N_CORES):
        shard = np.ascontiguousarray(
            x[core * NN_PER_CORE:(core + 1) * NN_PER_CORE].astype(np.float16)
        )
        in_maps.append({"x": shard, "wqkT": wqkT, "wvT": wvT, "wprojT": wprojT})

    kw = {}
    if _trace:
        import tempfile
        kw = dict(trace=True, tmpdir=tempfile.mkdtemp(prefix="attn2_trace_"))
    res = run_bass_kernel_spmd(nc, in_maps, list(range(N_CORES)), **kw)
    LAST_RESULT["res"] = res
    LAST_RESULT["tmpdir"] = kw.get("tmpdir")
    out = np.empty((n, c, t, vv), dtype=np.float32)
    for core in range(N_CORES):
        out[core * NN_PER_CORE:(core + 1) * NN_PER_CORE] = \
            res.results[core]["y"].astype(np.float32)
    return out
